# revision 14
# baseline (speedup 1.0000x reference)
"""Trainium2 Bass kernel for a 2-layer "BiGRU" (batch-flipped, per reference).

v2: same wavefront/bulk-gi structure as v1, but the per-step work is fused:
  * All four GRU cells' gate math runs as ONE op set on partitions 0:104
    (f0 rows 0:8, b0 32:40, f1 64:72, b1 96:104 — one shared PSUM tile).
  * One combined identity/selector inject per column block (covers both
    layers), instead of per-layer injects.
  * The per-step h->hT transposes write one PSUM tile (4 transposes), then
    TWO vector copies land the whole transposed state (+ the batch-reversed
    b0 view for layer-1's bulk input matmuls) in a single combined chunk
    buffer CB[128, k, step, 112] that directly feeds all recurrent lhsT
    slices and the bulk gi1 lhsT slices.

Self-contained: hardcodes all shapes from the problem spec.
"""

import numpy as np

from concourse import bacc, tile
from concourse.bass import mybir

SEQ, BATCH, IN, HID = 512, 64, 512, 512
G3 = 3 * HID  # 1536
BC = 8        # local batch per core
NCORES = 8
CH = 16       # wavefront chunk (steps) for layer-1 input bulk matmuls
LAG = 20      # layer-1 lag behind layer-0 (> CH + bulk spread, multiple of W4)
W4 = 2        # gi DMA window (steps)
FP32 = mybir.dt.float32
BF16 = mybir.dt.bfloat16

# cell name, PSUM base partition, input K-chunks of 128
CELLS = [("f0", 0, 4), ("b0", 32, 4), ("f1", 64, 8), ("b1", 96, 8)]


def _blob_layout():
    """Free-dim offsets (in bf16 elements) inside the single load blob."""
    off = {}
    cur = 0
    for cname, _, kx in CELLS:
        for k in range(kx):
            off[f"w_{cname}{k}"] = cur
            cur += G3
        for k in range(4):
            off[f"u_{cname}{k}"] = cur
            cur += G3
    off["bias"] = cur      # rows 0:4 = cells; per cell: [bulk_bias 1536 | bhn 512]
    cur += 2048
    off["ohrow"] = cur     # rows 0:4; cell c: cols c*128..+128 = 1.0 in row c
    cur += 4 * 128
    off["i104"] = cur      # 104x104 identity at partitions 0:104, rows 40:64 zero
    cur += 104
    off["bselall"] = cur   # (4, 104) bias-row selector covering all four bands
    cur += 104
    return off, cur


def build_core_program(S, repeats=1):
    assert S % CH == 0 and LAG % W4 == 0 and CH % W4 == 0
    nc = bacc.Bacc(None, target_bir_lowering=False)

    off, totw = _blob_layout()
    blob_d = nc.declare_dram_parameter("blob", [128, totw // 2], FP32, isOutput=False)
    xTp_d = nc.declare_dram_parameter("xTp", [128, 4, S * BC // 2], FP32, isOutput=False)
    out_d = nc.declare_dram_parameter("out", [S, 16, HID], BF16, isOutput=True)

    with tile.TileContext(nc) as tc:
        for _ in range(repeats):
            build_body(nc, tc, S, blob_d, xTp_d, out_d, off, totw)
    nc.compile()
    return nc


def build_body(nc, tc, S, blob_d, xTp_d, out_d, off, totw):
    import contextlib

    ACT = mybir.ActivationFunctionType
    OP = mybir.AluOpType
    NCHUNK = S // CH
    P104 = slice(0, 104)

    ctx = contextlib.ExitStack()
    with ctx:
        const = ctx.enter_context(tc.tile_pool(name="const", bufs=1))
        ghp = ctx.enter_context(tc.tile_pool(name="ghp", bufs=2, space="PSUM"))
        scr = ctx.enter_context(tc.tile_pool(name="scr", bufs=1, space="PSUM"))
        scrT = scr
        dram = ctx.enter_context(tc.tile_pool(name="dram", bufs=1, space="DRAM"))
        xr_pool = ctx.enter_context(tc.tile_pool(name="xr", bufs=2))
        ev_pool = ctx.enter_context(tc.tile_pool(name="ev", bufs=2))
        ring_pool = ctx.enter_context(tc.tile_pool(name="ring", bufs=3))
        cb_pool = ctx.enter_context(tc.tile_pool(name="cb", bufs=2))
        g_pool = ctx.enter_context(tc.tile_pool(name="g", bufs=3))

        # ---- load blob (single DMA), bf16 views via bitcast ----
        blob = const.tile([128, totw // 2], FP32, tag="blob")
        nc.gpsimd.dma_start(out=blob[:], in_=blob_d[:])
        b16 = blob[:].bitcast(BF16)

        W, U, BULKB, OHR = {}, {}, {}, {}
        ob, oh = off["bias"], off["ohrow"]
        for ci, (cname, base, kx) in enumerate(CELLS):
            W[cname] = [b16[:, off[f"w_{cname}{k}"]:off[f"w_{cname}{k}"] + G3]
                        for k in range(kx)]
            U[cname] = [b16[:, off[f"u_{cname}{k}"]:off[f"u_{cname}{k}"] + G3]
                        for k in range(4)]
            BULKB[cname] = b16[0:4, ob:ob + 1536]      # row ci is live
            OHR[cname] = b16[0:4, oh + ci * 128:oh + (ci + 1) * 128]
        BHNROWS = b16[0:4, ob + 1536:ob + 2048]
        oi = off["i104"]
        I104 = b16[0:104, oi:oi + 104]
        obs = off["bselall"]
        BSELALL = b16[0:4, obs:obs + 104]

        zero16 = const.tile([128, 512], BF16, tag="zero16")
        nc.any.memset(zero16[:], 0.0)

        # ---- internal DRAM for bulk gi results ----
        gi0_dram = {c: dram.tile([S * BC, G3], BF16, tag=f"gi0_{c}", name=f"gi0_{c}", uniquify=True)
                    for c in ("f0", "b0")}
        gi1_dram = {c: [dram.tile([CH * BC, G3], BF16, tag=f"gi1_{c}", bufs=4,
                                  name=f"gi1_{c}_{cc}")
                        for cc in range(NCHUNK)]
                    for c in ("f1", "b1")}

        def bulk_gi(cell, lhs_chunks, out_rows_ap):
            """One 128-row tile of gi = x @ wihT + bias -> DRAM (bf16)."""
            kx = len(lhs_chunks)
            for n in range(3):
                ps = scr.tile([128, 512], FP32, tag="scr", bufs=1)
                for k in range(kx):
                    nc.tensor.matmul(out=ps[:], lhsT=lhs_chunks[k],
                                     rhs=W[cell][k][:, n * 512:(n + 1) * 512],
                                     start=(k == 0), stop=False)
                nc.tensor.matmul(out=ps[:], lhsT=OHR[cell],
                                 rhs=BULKB[cell][:, n * 512:(n + 1) * 512],
                                 start=False, stop=True)
                ev = ev_pool.tile([128, 512], BF16, tag="ev")
                nc.vector.tensor_copy(out=ev[:], in_=ps[:])
                nc.gpsimd.dma_start(out=out_rows_ap[:, n * 512:(n + 1) * 512],
                                    in_=ev[:])

        # ---- prepass: gi0 for all steps ----
        for rt in range(S * BC // 128):
            xrt = xr_pool.tile([128, 4, 64], FP32, tag="xr")
            nc.gpsimd.dma_start(out=xrt[:], in_=xTp_d[:, :, rt * 64:(rt + 1) * 64])
            x16 = xrt[:].bitcast(BF16)   # (128, 4, 128)
            for cell in ("f0", "b0"):
                bulk_gi(cell, [x16[:, k, :] for k in range(4)],
                        gi0_dram[cell][rt * 128:(rt + 1) * 128, :])

        # ---- wavefront loop: L0 at step i, L1 at step i-LAG, all four
        # cells sharing one PSUM tile and one gate-op chain ----
        CBs, BAs, BCs, ghs = {}, {}, {}, {}
        hprev = zero16[:, 0:512]

        def dma_ring(iw):
            """Prefetch one W4-step window of gi slices for both layers."""
            t0w, t1w = iw, iw - LAG
            r = ring_pool.tile([128, W4 * G3], BF16, tag="ring",
                               name=f"ring{iw}")
            if iw < 3 * W4:
                # first touch of each ring slot: zero everything so dead
                # rows stay finite-zero and pre-LAG L1 gi reads are zero
                nc.any.memset(r[:], 0.0)
            rv = r.rearrange("p (s g) -> p s g", s=W4)
            if 0 <= t0w < S:
                for cell, base in (("f0", 0), ("b0", 32)):
                    src = gi0_dram[cell][:].rearrange(
                        "(s b) g -> b s g", b=BC)[:, t0w:t0w + W4, :]
                    nc.sync.dma_start(out=rv[base:base + BC], in_=src)
            if 0 <= t1w < S:
                for cell, base in (("f1", 64), ("b1", 96)):
                    src = gi1_dram[cell][t1w // CH][:].rearrange(
                        "(s b) g -> b s g", b=BC)[:, t1w % CH:t1w % CH + W4, :]
                    nc.sync.dma_start(out=rv[base:base + BC], in_=src)
            return r

        ring_next = dma_ring(0)
        ring = None
        for i in range(S + LAG):
            t0, t1 = i, i - LAG

            if i % W4 == 0:
                ring = ring_next
                ring_next = dma_ring(i + W4) if i + W4 < S + LAG else None
            wi = i % W4

            if i % CH == 0:
                CBs[i // CH] = cb_pool.tile([128, 4, CH, 104], BF16, tag="cb",
                                            name=f"cb{i // CH}")
                BAs[i // CH] = cb_pool.tile([128, 4, CH * BC], BF16, tag="ba",
                                            name=f"ba{i // CH}")
                BCs[i // CH] = cb_pool.tile([128, 4, CH * BC], BF16, tag="bc",
                                            name=f"bc{i // CH}")

            def hT(col, k):
                cbp = CBs[(i - 1) // CH]
                return cbp[:, k, (i - 1) % CH, col:col + 8]

            def emit_injects(j, ringj, wj):
                # one combined inject per column block covers both layers'
                # bands (dead rows get 0); emitted one iteration EARLY so
                # the PE fills its gate-tail idle gap with them
                ghj = ghp.tile([128, G3], FP32, tag="gh", bufs=2,
                               name=f"gh_{j}")
                ghs[j] = ghj
                for col, inj in ((0, "gi"), (1024, "bhn"), (512, "gi")):
                    if inj == "gi":
                        nc.tensor.matmul(
                            out=ghj[0:104, col:col + 512], lhsT=I104,
                            rhs=ringj[0:104, wj * G3 + col:wj * G3 + col + 512],
                            start=True, stop=(j == 0), tile_position=(0, 0))
                    else:
                        nc.tensor.matmul(
                            out=ghj[0:104, col:col + 512], lhsT=BSELALL,
                            rhs=BHNROWS, start=True, stop=(j == 0),
                            tile_position=(0, 0))

            if i == 0:
                emit_injects(0, ring, 0)
            gh = ghs.pop(i)
            # column blocks in dependency order r, n, z
            for col in (0, 1024, 512):
                for k in range(4):
                    if 1 <= t0 < S:
                        for cell, base in (("f0", 0), ("b0", 32)):
                            nc.tensor.matmul(
                                out=gh[base:base + 8, col:col + 512],
                                lhsT=hT(base, k),
                                rhs=U[cell][k][:, col:col + 512],
                                start=False, stop=(k == 3),
                                tile_position=(0, base))
                    if 1 <= t1 < S:
                        for cell, base in (("f1", 64), ("b1", 96)):
                            nc.tensor.matmul(
                                out=gh[base:base + 8, col:col + 512],
                                lhsT=hT(base, k),
                                rhs=U[cell][k][:, col:col + 512],
                                start=False, stop=(k == 3),
                                tile_position=(0, base))
            if i + 1 < S + LAG:
                if (i + 1) % W4 == 0:
                    emit_injects(i + 1, ring_next, 0)
                else:
                    emit_injects(i + 1, ring, (i + 1) % W4)

            # ---------- shared gates: one op chain for all four cells ----
            rz = g_pool.tile([128, 1024], BF16, tag="rz")
            av = g_pool.tile([128, 512], BF16, tag="av")
            bv = g_pool.tile([128, 512], BF16, tag="bv")
            nv = g_pool.tile([128, 512], BF16, tag="nv")
            dv = g_pool.tile([128, 512], BF16, tag="dv")
            ev2 = g_pool.tile([128, 512], BF16, tag="ev2")
            h2 = g_pool.tile([128, 512], BF16, tag="h2")
            nc.scalar.activation(rz[P104, 0:512], gh[P104, 0:512], ACT.Sigmoid)
            nc.vector.tensor_tensor(out=av[P104, :], in0=rz[P104, 0:512],
                                    in1=gh[P104, 1024:1536], op=OP.mult)
            nc.vector.tensor_tensor(
                out=bv[P104, :], in0=av[P104, :],
                in1=ring[P104, wi * G3 + 1024:wi * G3 + 1536], op=OP.add)
            nc.scalar.activation(nv[P104, :], bv[P104, :], ACT.Tanh)
            if i == LAG:
                # layer-1's first live step: its h_prev must be zero, the
                # pre-LAG band contents are garbage
                nc.vector.tensor_tensor(out=dv[0:40, :], in0=hprev[0:40, :],
                                        in1=nv[0:40, :], op=OP.subtract)
                nc.vector.tensor_tensor(out=dv[64:104, :],
                                        in0=zero16[64:104, 0:512],
                                        in1=nv[64:104, :], op=OP.subtract)
            else:
                nc.vector.tensor_tensor(out=dv[P104, :], in0=hprev[P104, :],
                                        in1=nv[P104, :], op=OP.subtract)

            # ---------- z-tail in hidden halves: sig_z -> e -> h2 ->
            # transpose -> CB copy per half, so the next step's k=0,1
            # stationary chunks release one half earlier ----------
            Tt = scrT.tile([128, 416], BF16, tag="T", bufs=1, name=f"T_{i}")
            cb = CBs[i // CH]
            tt = i % CH
            sl = tt * BC
            Tv = Tt[:].rearrange("p (c w) -> p c w", c=4)
            for hf in (0, 1):
                cs = slice(hf * 256, hf * 256 + 256)
                zs = slice(512 + hf * 256, 768 + hf * 256)
                nc.scalar.activation(rz[P104, zs], gh[P104, zs], ACT.Sigmoid)
                nc.vector.tensor_tensor(out=ev2[P104, cs], in0=rz[P104, zs],
                                        in1=dv[P104, cs], op=OP.mult)
                nc.vector.tensor_tensor(out=h2[P104, cs], in0=nv[P104, cs],
                                        in1=ev2[P104, cs], op=OP.add)
                for c in (2 * hf, 2 * hf + 1):
                    nc.tensor.transpose(out=Tt[:, c * 104:(c + 1) * 104],
                                        in_=h2[0:104, c * 128:(c + 1) * 128],
                                        identity=I104)
                nc.vector.tensor_copy(
                    out=cb[:, 2 * hf:2 * hf + 2, tt, 0:104],
                    in_=Tv[:, 2 * hf:2 * hf + 2, :])
            hprev = h2[:, 0:512]

            if 0 <= t1 < S:
                # only the 16 live rows (f1 at 64:72, b1 at 96:104)
                nc.sync.dma_start(out=out_d[t1, 0:8, :], in_=h2[64:72, :])
                nc.sync.dma_start(out=out_d[t1, 8:16, :], in_=h2[96:104, :])
            # bulk-input copies are off the critical path
            nc.vector.tensor_copy(out=BAs[i // CH][:, :, sl:sl + BC],
                                  in_=Tv[:, :, 0:8])
            nc.vector.tensor_copy(out=BCs[i // CH][:, :, sl:sl + BC],
                                  in_=Tv[:, :, 39:31:-1])

            # ---------- bulk gi1, spread one n-slice group per iteration ----
            bc = (t0 - (CH - 1)) // CH          # chunk completed CH-1 iters ago
            ph = (t0 - (CH - 1)) % CH
            if 0 <= bc < NCHUNK and ph < 3:
                lhs = [BAs[bc][:, k, :] for k in range(4)] + \
                      [BCs[bc][:, k, :] for k in range(4)]
                for gidx in (ph * 2, ph * 2 + 1):
                    cell = ("f1", "b1")[gidx // 3]
                    n = gidx % 3
                    ps = scr.tile([128, 512], FP32, tag="scr", bufs=1)
                    for k in range(8):
                        nc.tensor.matmul(out=ps[:], lhsT=lhs[k],
                                         rhs=W[cell][k][:, n * 512:(n + 1) * 512],
                                         start=(k == 0), stop=False)
                    nc.tensor.matmul(out=ps[:], lhsT=OHR[cell],
                                     rhs=BULKB[cell][:, n * 512:(n + 1) * 512],
                                     start=False, stop=True)
                    ev = ev_pool.tile([128, 512], BF16, tag="ev")
                    nc.vector.tensor_copy(out=ev[:], in_=ps[:])
                    nc.gpsimd.dma_start(
                        out=gi1_dram[cell][bc][:, n * 512:(n + 1) * 512], in_=ev[:])




# ---------------------------------------------------------------------------
# host side
# ---------------------------------------------------------------------------

_CACHE = {}


def _groups():
    return [list(range(4 * d, 4 * d + 4)) + [63 - (4 * d + 3), 63 - (4 * d + 2),
            63 - (4 * d + 1), 63 - 4 * d] for d in range(NCORES)]


def _bf16_u16(a):
    a = np.ascontiguousarray(a, np.float32)
    u = a.view(np.uint32)
    return ((u + 0x7FFF + ((u >> 16) & 1)) >> 16).astype(np.uint16)


def _pack_words(u16):
    ev = u16[..., 0::2].astype(np.uint32)
    od = u16[..., 1::2].astype(np.uint32)
    return (ev | (od << 16)).view(np.float32)


def _blob_host(inputs):
    off, totw = _blob_layout()
    blob = np.zeros((128, totw), np.uint16)
    for ci, (cname, base, kx) in enumerate(CELLS):
        wih = np.asarray(inputs[f"wih_{cname}"], np.float32)   # (1536, in)
        whh = np.asarray(inputs[f"whh_{cname}"], np.float32)   # (1536, 512)
        bih = np.asarray(inputs[f"bih_{cname}"], np.float32)
        bhh = np.asarray(inputs[f"bhh_{cname}"], np.float32)
        wt = _bf16_u16(wih.T.reshape(kx, 128, G3))
        ut = _bf16_u16(whh.T.reshape(4, 128, G3))
        for k in range(kx):
            o = off[f"w_{cname}{k}"]
            blob[:, o:o + G3] = wt[k]
        for k in range(4):
            o = off[f"u_{cname}{k}"]
            blob[:, o:o + G3] = ut[k]
        bulkb = np.concatenate([(bih + bhh)[:1024], bih[1024:]])
        blob[ci, off["bias"]:off["bias"] + 1536] = _bf16_u16(bulkb)
        blob[ci, off["bias"] + 1536:off["bias"] + 2048] = _bf16_u16(bhh[1024:])
        blob[ci, off["ohrow"] + ci * 128:off["ohrow"] + (ci + 1) * 128] = \
            _bf16_u16(np.ones(128, np.float32))
    one = _bf16_u16(np.ones(1, np.float32))[0]
    for j in range(104):
        if not (40 <= j < 64):
            blob[j, off["i104"] + j] = one
    for r, a, b in ((0, 0, 8), (1, 32, 40), (2, 64, 72), (3, 96, 104)):
        blob[r, off["bselall"] + a:off["bselall"] + b] = one
    return _pack_words(blob)


def _in_maps(inputs):
    S = inputs["x"].shape[0]
    x = np.asarray(inputs["x"], np.float32)
    groups = _groups()
    blob = _blob_host(inputs)
    in_maps = []
    for d in range(NCORES):
        xl = x[:, groups[d], :]                       # (S, 8, 512)
        # xTp layout: (128 part, 4 k, S*BC) -> words
        xT = _bf16_u16(xl.transpose(2, 0, 1).reshape(4, 128, S * BC))
        xT = np.ascontiguousarray(xT.transpose(1, 0, 2))   # (128, 4, S*BC)
        in_maps.append({"blob": blob, "xTp": _pack_words(xT)})
    return in_maps


def _assemble(outs, S):
    groups = _groups()
    out = np.zeros((S, BATCH, 2 * HID), np.float32)
    for d in range(NCORES):
        raw = np.asarray(outs[d]["out"], np.float32)  # (S, 16, 512)
        G = groups[d]
        for b in range(BC):
            out[:, G[b], 0:HID] = raw[:, b, :]
            out[:, G[b], HID:] = raw[:, 8 + 7 - b, :]
    return out


class _Runner:
    """Caches the traced+compiled SPMD executable so repeat calls skip the
    (expensive) jax retrace and BIR re-serialization."""

    def __init__(self, S):
        import jax
        from jax.sharding import Mesh, PartitionSpec
        from jax.experimental.shard_map import shard_map
        from concourse import bass2jax
        from concourse.bass2jax import _bass_exec_p, partition_id_tensor

        bass2jax.install_neuronx_cc_hook()
        self.S = S
        nc = build_core_program(S)
        self.nc = nc
        partition_name = nc.partition_id_tensor.name if nc.partition_id_tensor else None
        in_names, out_names, out_avals, zero_outs = [], [], [], []
        for alloc in nc.m.functions[0].allocations:
            if not isinstance(alloc, mybir.MemoryLocationSet):
                continue
            name = alloc.memorylocations[0].name
            if alloc.kind == "ExternalInput":
                if name != partition_name:
                    in_names.append(name)
            elif alloc.kind == "ExternalOutput":
                shape = tuple(alloc.tensor_shape)
                dtype = mybir.dt.np(alloc.dtype)
                out_names.append(name)
                out_avals.append(jax.core.ShapedArray(shape, dtype))
                zero_outs.append(np.zeros(shape, dtype))
        n_params = len(in_names)
        self.in_names = list(in_names)
        self.out_names = out_names
        self.out_shapes = [tuple(a.shape) for a in out_avals]
        self.zero_outs = zero_outs
        all_in = in_names + out_names + ([partition_name] if partition_name else [])

        def _body(*args):
            operands = list(args)
            if partition_name is not None:
                operands.append(partition_id_tensor())
            return tuple(_bass_exec_p.bind(
                *operands,
                out_avals=tuple(out_avals),
                in_names=tuple(all_in),
                out_names=tuple(out_names),
                lowering_input_output_aliases=(),
                sim_require_finite=True,
                sim_require_nnan=True,
                nc=nc,
            ))

        devices = jax.devices()[:NCORES]
        mesh = Mesh(np.asarray(devices), ("core",))
        in_specs = (PartitionSpec("core"),) * (n_params + len(out_names))
        out_specs = (PartitionSpec("core"),) * len(out_names)
        self.fn = jax.jit(
            shard_map(_body, mesh=mesh, in_specs=in_specs,
                      out_specs=out_specs, check_rep=False),
            keep_unused=True)
        self.jax = jax

    def _zero_dev(self):
        # output buffers are fully overwritten by the kernel, so one
        # device-resident zero set can be reused across calls
        if not hasattr(self, "_zero_dev_bufs"):
            self._zero_dev_bufs = [
                self.jax.device_put(
                    np.zeros((NCORES * z.shape[0], *z.shape[1:]), z.dtype))
                for z in self.zero_outs]
        return self._zero_dev_bufs

    def run(self, in_maps):
        concat_in = [
            np.concatenate([np.asarray(m[nm]) for m in in_maps], axis=0)
            for nm in self.in_names]
        outs = self.fn(*concat_in, *self._zero_dev())
        return [
            {nm: np.asarray(outs[i]).reshape(NCORES, *self.out_shapes[i])[c]
             for i, nm in enumerate(self.out_names)}
            for c in range(NCORES)]

    def run_timed(self, in_maps, iters=5):
        """Stage inputs (and the pre-zeroed output buffers — every output
        element is written, so reuse is safe) on device; time executions."""
        import time
        concat_in = [
            self.jax.device_put(np.concatenate(
                [np.asarray(m[nm]) for m in in_maps], axis=0))
            for nm in self.in_names]
        concat_zero = self._zero_dev()
        o = self.fn(*concat_in, *concat_zero)
        self.jax.block_until_ready(o)
        best = float("inf")
        for _ in range(iters):
            t0 = time.perf_counter()
            o = self.fn(*concat_in, *concat_zero)
            self.jax.block_until_ready(o)
            best = min(best, time.perf_counter() - t0)
        return best


def kernel(**inputs):
    S = inputs["x"].shape[0]
    if S not in _CACHE:
        _CACHE[S] = _Runner(S)
    runner = _CACHE[S]
    outs = runner.run(_in_maps(inputs))
    return _assemble(outs, S)


if __name__ == "__main__":
    rng = np.random.default_rng(0)
    S = 32
    inputs = {"x": rng.standard_normal((S, 64, 512), dtype=np.float32)}
    s = 1.0 / np.sqrt(HID)
    u = lambda *shp: rng.uniform(-s, s, shp).astype(np.float32)
    for c, idim in (("f0", 512), ("b0", 512), ("f1", 1024), ("b1", 1024)):
        inputs[f"wih_{c}"] = u(G3, idim)
        inputs[f"whh_{c}"] = u(G3, HID)
        inputs[f"bih_{c}"] = u(G3)
        inputs[f"bhh_{c}"] = u(G3)
    out = kernel(**inputs)
    print("kernel ran, out", out.shape, float(np.abs(out).mean()))


# revision 15
# speedup vs baseline: 1.0761x; 1.0761x over previous
"""Trainium2 Bass kernel for a 2-layer "BiGRU" (batch-flipped, per reference).

v2: same wavefront/bulk-gi structure as v1, but the per-step work is fused:
  * All four GRU cells' gate math runs as ONE op set on partitions 0:104
    (f0 rows 0:8, b0 32:40, f1 64:72, b1 96:104 — one shared PSUM tile).
  * One combined identity/selector inject per column block (covers both
    layers), instead of per-layer injects.
  * The per-step h->hT transposes write one PSUM tile (4 transposes), then
    TWO vector copies land the whole transposed state (+ the batch-reversed
    b0 view for layer-1's bulk input matmuls) in a single combined chunk
    buffer CB[128, k, step, 112] that directly feeds all recurrent lhsT
    slices and the bulk gi1 lhsT slices.

Self-contained: hardcodes all shapes from the problem spec.
"""

import numpy as np

from concourse import bacc, tile
from concourse.bass import mybir

SEQ, BATCH, IN, HID = 512, 64, 512, 512
G3 = 3 * HID  # 1536
BC = 8        # local batch per core
NCORES = 8
CH = 16       # wavefront chunk (steps) for layer-1 input bulk matmuls
LAG = 20      # layer-1 lag behind layer-0 (> CH + bulk spread, multiple of W4)
W4 = 2        # gi DMA window (steps)
FP32 = mybir.dt.float32
BF16 = mybir.dt.bfloat16

# cell name, PSUM base partition, input K-chunks of 128
CELLS = [("f0", 0, 4), ("b0", 32, 4), ("f1", 64, 8), ("b1", 96, 8)]


def _blob_layout():
    """Free-dim offsets (in bf16 elements) inside the single load blob."""
    off = {}
    cur = 0
    for cname, _, kx in CELLS:
        for k in range(kx):
            off[f"w_{cname}{k}"] = cur
            cur += G3
        for k in range(4):
            off[f"u_{cname}{k}"] = cur
            cur += G3
    off["bias"] = cur      # rows 0:4 = cells; per cell: [bulk_bias 1536 | bhn 512]
    cur += 2048
    off["ohrow"] = cur     # rows 0:4; cell c: cols c*128..+128 = 1.0 in row c
    cur += 4 * 128
    off["i104"] = cur      # 104x104 identity at partitions 0:104, rows 40:64 zero
    cur += 104
    off["bselall"] = cur   # (4, 104) bias-row selector covering all four bands
    cur += 104
    return off, cur


def build_core_program(S, repeats=1):
    assert S % CH == 0 and LAG % W4 == 0 and CH % W4 == 0
    nc = bacc.Bacc(None, target_bir_lowering=False)

    off, totw = _blob_layout()
    blob_d = nc.declare_dram_parameter("blob", [128, totw // 2], FP32, isOutput=False)
    xTp_d = nc.declare_dram_parameter("xTp", [128, 4, S * BC // 2], FP32, isOutput=False)
    out_d = nc.declare_dram_parameter("out", [S, 16, HID], BF16, isOutput=True)

    with tile.TileContext(nc) as tc:
        for _ in range(repeats):
            build_body(nc, tc, S, blob_d, xTp_d, out_d, off, totw)
    nc.compile()
    return nc


def build_body(nc, tc, S, blob_d, xTp_d, out_d, off, totw):
    import contextlib

    ACT = mybir.ActivationFunctionType
    OP = mybir.AluOpType
    NCHUNK = S // CH
    P104 = slice(0, 104)

    ctx = contextlib.ExitStack()
    with ctx:
        const = ctx.enter_context(tc.tile_pool(name="const", bufs=1))
        ghp = ctx.enter_context(tc.tile_pool(name="ghp", bufs=2, space="PSUM"))
        scr = ctx.enter_context(tc.tile_pool(name="scr", bufs=1, space="PSUM"))
        scrT = scr
        dram = ctx.enter_context(tc.tile_pool(name="dram", bufs=1, space="DRAM"))
        xr_pool = ctx.enter_context(tc.tile_pool(name="xr", bufs=2))
        ev_pool = ctx.enter_context(tc.tile_pool(name="ev", bufs=2))
        ring_pool = ctx.enter_context(tc.tile_pool(name="ring", bufs=3))
        cb_pool = ctx.enter_context(tc.tile_pool(name="cb", bufs=2))
        g_pool = ctx.enter_context(tc.tile_pool(name="g", bufs=3))

        # ---- load blob (single DMA), bf16 views via bitcast ----
        blob = const.tile([128, totw // 2], FP32, tag="blob")
        nc.gpsimd.dma_start(out=blob[:], in_=blob_d[:])
        b16 = blob[:].bitcast(BF16)

        W, U, BULKB, OHR = {}, {}, {}, {}
        ob, oh = off["bias"], off["ohrow"]
        for ci, (cname, base, kx) in enumerate(CELLS):
            W[cname] = [b16[:, off[f"w_{cname}{k}"]:off[f"w_{cname}{k}"] + G3]
                        for k in range(kx)]
            U[cname] = [b16[:, off[f"u_{cname}{k}"]:off[f"u_{cname}{k}"] + G3]
                        for k in range(4)]
            BULKB[cname] = b16[0:4, ob:ob + 1536]      # row ci is live
            OHR[cname] = b16[0:4, oh + ci * 128:oh + (ci + 1) * 128]
        BHNROWS = b16[0:4, ob + 1536:ob + 2048]
        oi = off["i104"]
        I104 = b16[0:104, oi:oi + 104]
        obs = off["bselall"]
        BSELALL = b16[0:4, obs:obs + 104]

        zero16 = const.tile([128, 512], BF16, tag="zero16")
        nc.any.memset(zero16[:], 0.0)

        # ---- internal DRAM for bulk gi results ----
        gi0_dram = {c: dram.tile([S * BC, G3], BF16, tag=f"gi0_{c}", name=f"gi0_{c}", uniquify=True)
                    for c in ("f0", "b0")}
        gi1_dram = {c: [dram.tile([CH * BC, G3], BF16, tag=f"gi1_{c}", bufs=4,
                                  name=f"gi1_{c}_{cc}")
                        for cc in range(NCHUNK)]
                    for c in ("f1", "b1")}

        def bulk_gi(cell, lhs_chunks, out_rows_ap):
            """One 128-row tile of gi = x @ wihT + bias -> DRAM (bf16)."""
            kx = len(lhs_chunks)
            for n in range(3):
                ps = scr.tile([128, 512], FP32, tag="scr", bufs=1)
                for k in range(kx):
                    nc.tensor.matmul(out=ps[:], lhsT=lhs_chunks[k],
                                     rhs=W[cell][k][:, n * 512:(n + 1) * 512],
                                     start=(k == 0), stop=False)
                nc.tensor.matmul(out=ps[:], lhsT=OHR[cell],
                                 rhs=BULKB[cell][:, n * 512:(n + 1) * 512],
                                 start=False, stop=True)
                ev = ev_pool.tile([128, 512], BF16, tag="ev")
                nc.vector.tensor_copy(out=ev[:], in_=ps[:])
                nc.gpsimd.dma_start(out=out_rows_ap[:, n * 512:(n + 1) * 512],
                                    in_=ev[:])

        # ---- prepass: gi0 for all steps ----
        for rt in range(S * BC // 128):
            xrt = xr_pool.tile([128, 4, 64], FP32, tag="xr")
            nc.gpsimd.dma_start(out=xrt[:], in_=xTp_d[:, :, rt * 64:(rt + 1) * 64])
            x16 = xrt[:].bitcast(BF16)   # (128, 4, 128)
            for cell in ("f0", "b0"):
                bulk_gi(cell, [x16[:, k, :] for k in range(4)],
                        gi0_dram[cell][rt * 128:(rt + 1) * 128, :])

        # ---- wavefront loop: L0 at step i, L1 at step i-LAG, all four
        # cells sharing one PSUM tile and one gate-op chain ----
        CBs, BAs, BCs, ghs = {}, {}, {}, {}
        hprev = zero16[:, 0:512]

        def dma_ring(iw):
            """Prefetch one W4-step window of gi slices for both layers."""
            t0w, t1w = iw, iw - LAG
            r = ring_pool.tile([128, W4 * G3], BF16, tag="ring",
                               name=f"ring{iw}")
            if iw < 3 * W4:
                # first touch of each ring slot: zero everything so dead
                # rows stay finite-zero and pre-LAG L1 gi reads are zero
                nc.any.memset(r[:], 0.0)
            rv = r.rearrange("p (s g) -> p s g", s=W4)
            if 0 <= t0w < S:
                for cell, base in (("f0", 0), ("b0", 32)):
                    src = gi0_dram[cell][:].rearrange(
                        "(s b) g -> b s g", b=BC)[:, t0w:t0w + W4, :]
                    nc.sync.dma_start(out=rv[base:base + BC], in_=src)
            if 0 <= t1w < S:
                for cell, base in (("f1", 64), ("b1", 96)):
                    src = gi1_dram[cell][t1w // CH][:].rearrange(
                        "(s b) g -> b s g", b=BC)[:, t1w % CH:t1w % CH + W4, :]
                    nc.sync.dma_start(out=rv[base:base + BC], in_=src)
            return r

        ring_next = dma_ring(0)
        ring = None
        for i in range(S + LAG):
            t0, t1 = i, i - LAG

            if i % W4 == 0:
                ring = ring_next
                ring_next = dma_ring(i + W4) if i + W4 < S + LAG else None
            wi = i % W4

            if i % CH == 0:
                CBs[i // CH] = cb_pool.tile([128, 4, CH, 104], BF16, tag="cb",
                                            name=f"cb{i // CH}")
                BAs[i // CH] = cb_pool.tile([128, 4, CH * BC], BF16, tag="ba",
                                            name=f"ba{i // CH}")
                BCs[i // CH] = cb_pool.tile([128, 4, CH * BC], BF16, tag="bc",
                                            name=f"bc{i // CH}")

            def hT(col, k):
                cbp = CBs[(i - 1) // CH]
                return cbp[:, k, (i - 1) % CH, col:col + 8]

            def emit_injects(j, ringj, wj):
                # one combined inject per column block covers both layers'
                # bands (dead rows get 0); emitted one iteration EARLY so
                # the PE fills its gate-tail idle gap with them
                ghj = ghp.tile([128, G3], FP32, tag="gh", bufs=2,
                               name=f"gh_{j}")
                ghs[j] = ghj
                for col, inj in ((0, "gi"), (1024, "bhn"), (512, "gi")):
                    if inj == "gi":
                        nc.tensor.matmul(
                            out=ghj[0:104, col:col + 512], lhsT=I104,
                            rhs=ringj[0:104, wj * G3 + col:wj * G3 + col + 512],
                            start=True, stop=(j == 0), tile_position=(0, 0))
                    else:
                        nc.tensor.matmul(
                            out=ghj[0:104, col:col + 512], lhsT=BSELALL,
                            rhs=BHNROWS, start=True, stop=(j == 0),
                            tile_position=(0, 0))

            if i == 0:
                emit_injects(0, ring, 0)
            gh = ghs.pop(i)
            # column blocks in dependency order r, n, z
            for col in (0, 1024, 512):
                for k in range(4):
                    if 1 <= t0 < S:
                        for cell, base in (("f0", 0), ("b0", 32)):
                            nc.tensor.matmul(
                                out=gh[base:base + 8, col:col + 512],
                                lhsT=hT(base, k),
                                rhs=U[cell][k][:, col:col + 512],
                                start=False, stop=(k == 3),
                                tile_position=(0, base))
                    if 1 <= t1 < S:
                        for cell, base in (("f1", 64), ("b1", 96)):
                            nc.tensor.matmul(
                                out=gh[base:base + 8, col:col + 512],
                                lhsT=hT(base, k),
                                rhs=U[cell][k][:, col:col + 512],
                                start=False, stop=(k == 3),
                                tile_position=(0, base))
            if i + 1 < S + LAG:
                if (i + 1) % W4 == 0:
                    emit_injects(i + 1, ring_next, 0)
                else:
                    emit_injects(i + 1, ring, (i + 1) % W4)

            # ---------- shared gates: one op chain for all four cells ----
            rz = g_pool.tile([128, 1024], BF16, tag="rz")
            av = g_pool.tile([128, 512], BF16, tag="av")
            bv = g_pool.tile([128, 512], BF16, tag="bv")
            nv = g_pool.tile([128, 512], BF16, tag="nv")
            dv = g_pool.tile([128, 512], BF16, tag="dv")
            ev2 = g_pool.tile([128, 512], BF16, tag="ev2")
            h2 = g_pool.tile([128, 512], BF16, tag="h2")
            nc.scalar.activation(rz[P104, 0:512], gh[P104, 0:512], ACT.Sigmoid)
            nc.vector.tensor_tensor(out=av[P104, :], in0=rz[P104, 0:512],
                                    in1=gh[P104, 1024:1536], op=OP.mult)
            nc.vector.tensor_tensor(
                out=bv[P104, :], in0=av[P104, :],
                in1=ring[P104, wi * G3 + 1024:wi * G3 + 1536], op=OP.add)
            nc.scalar.activation(nv[P104, :], bv[P104, :], ACT.Tanh)
            if i == LAG:
                # layer-1's first live step: its h_prev must be zero, the
                # pre-LAG band contents are garbage
                nc.vector.tensor_tensor(out=dv[0:40, :], in0=hprev[0:40, :],
                                        in1=nv[0:40, :], op=OP.subtract)
                nc.vector.tensor_tensor(out=dv[64:104, :],
                                        in0=zero16[64:104, 0:512],
                                        in1=nv[64:104, :], op=OP.subtract)
            else:
                nc.vector.tensor_tensor(out=dv[P104, :], in0=hprev[P104, :],
                                        in1=nv[P104, :], op=OP.subtract)

            # ---------- z-tail in hidden QUARTERS (one k-chunk each):
            # sig_z -> e -> h2 -> transpose -> CB copy per quarter, so the
            # next step's k=q stationary chunk releases as early as
            # possible and its matmul stream covers the later quarters ----
            Tt = scrT.tile([128, 416], BF16, tag="T", bufs=1, name=f"T_{i}")
            cb = CBs[i // CH]
            tt = i % CH
            sl = tt * BC
            Tv = Tt[:].rearrange("p (c w) -> p c w", c=4)
            for q in range(4):
                cs = slice(q * 128, q * 128 + 128)
                zs = slice(512 + q * 128, 640 + q * 128)
                nc.scalar.activation(rz[P104, zs], gh[P104, zs], ACT.Sigmoid)
                nc.vector.tensor_tensor(out=ev2[P104, cs], in0=rz[P104, zs],
                                        in1=dv[P104, cs], op=OP.mult)
                nc.vector.tensor_tensor(out=h2[P104, cs], in0=nv[P104, cs],
                                        in1=ev2[P104, cs], op=OP.add)
                nc.tensor.transpose(out=Tt[:, q * 104:(q + 1) * 104],
                                    in_=h2[0:104, q * 128:(q + 1) * 128],
                                    identity=I104)
                nc.vector.tensor_copy(out=cb[:, q, tt, 0:104],
                                      in_=Tv[:, q, :])
            hprev = h2[:, 0:512]

            if 0 <= t1 < S:
                # only the 16 live rows (f1 at 64:72, b1 at 96:104)
                nc.sync.dma_start(out=out_d[t1, 0:8, :], in_=h2[64:72, :])
                nc.sync.dma_start(out=out_d[t1, 8:16, :], in_=h2[96:104, :])
            # bulk-input copies are off the critical path
            nc.vector.tensor_copy(out=BAs[i // CH][:, :, sl:sl + BC],
                                  in_=Tv[:, :, 0:8])
            nc.vector.tensor_copy(out=BCs[i // CH][:, :, sl:sl + BC],
                                  in_=Tv[:, :, 39:31:-1])

            # ---------- bulk gi1, spread one n-slice group per iteration ----
            bc = (t0 - (CH - 1)) // CH          # chunk completed CH-1 iters ago
            ph = (t0 - (CH - 1)) % CH
            if 0 <= bc < NCHUNK and ph < 3:
                lhs = [BAs[bc][:, k, :] for k in range(4)] + \
                      [BCs[bc][:, k, :] for k in range(4)]
                for gidx in (ph * 2, ph * 2 + 1):
                    cell = ("f1", "b1")[gidx // 3]
                    n = gidx % 3
                    ps = scr.tile([128, 512], FP32, tag="scr", bufs=1)
                    for k in range(8):
                        nc.tensor.matmul(out=ps[:], lhsT=lhs[k],
                                         rhs=W[cell][k][:, n * 512:(n + 1) * 512],
                                         start=(k == 0), stop=False)
                    nc.tensor.matmul(out=ps[:], lhsT=OHR[cell],
                                     rhs=BULKB[cell][:, n * 512:(n + 1) * 512],
                                     start=False, stop=True)
                    ev = ev_pool.tile([128, 512], BF16, tag="ev")
                    nc.vector.tensor_copy(out=ev[:], in_=ps[:])
                    nc.gpsimd.dma_start(
                        out=gi1_dram[cell][bc][:, n * 512:(n + 1) * 512], in_=ev[:])




# ---------------------------------------------------------------------------
# host side
# ---------------------------------------------------------------------------

_CACHE = {}


def _groups():
    return [list(range(4 * d, 4 * d + 4)) + [63 - (4 * d + 3), 63 - (4 * d + 2),
            63 - (4 * d + 1), 63 - 4 * d] for d in range(NCORES)]


def _bf16_u16(a):
    a = np.ascontiguousarray(a, np.float32)
    u = a.view(np.uint32)
    return ((u + 0x7FFF + ((u >> 16) & 1)) >> 16).astype(np.uint16)


def _pack_words(u16):
    ev = u16[..., 0::2].astype(np.uint32)
    od = u16[..., 1::2].astype(np.uint32)
    return (ev | (od << 16)).view(np.float32)


def _blob_host(inputs):
    off, totw = _blob_layout()
    blob = np.zeros((128, totw), np.uint16)
    for ci, (cname, base, kx) in enumerate(CELLS):
        wih = np.asarray(inputs[f"wih_{cname}"], np.float32)   # (1536, in)
        whh = np.asarray(inputs[f"whh_{cname}"], np.float32)   # (1536, 512)
        bih = np.asarray(inputs[f"bih_{cname}"], np.float32)
        bhh = np.asarray(inputs[f"bhh_{cname}"], np.float32)
        wt = _bf16_u16(wih.T.reshape(kx, 128, G3))
        ut = _bf16_u16(whh.T.reshape(4, 128, G3))
        for k in range(kx):
            o = off[f"w_{cname}{k}"]
            blob[:, o:o + G3] = wt[k]
        for k in range(4):
            o = off[f"u_{cname}{k}"]
            blob[:, o:o + G3] = ut[k]
        bulkb = np.concatenate([(bih + bhh)[:1024], bih[1024:]])
        blob[ci, off["bias"]:off["bias"] + 1536] = _bf16_u16(bulkb)
        blob[ci, off["bias"] + 1536:off["bias"] + 2048] = _bf16_u16(bhh[1024:])
        blob[ci, off["ohrow"] + ci * 128:off["ohrow"] + (ci + 1) * 128] = \
            _bf16_u16(np.ones(128, np.float32))
    one = _bf16_u16(np.ones(1, np.float32))[0]
    for j in range(104):
        if not (40 <= j < 64):
            blob[j, off["i104"] + j] = one
    for r, a, b in ((0, 0, 8), (1, 32, 40), (2, 64, 72), (3, 96, 104)):
        blob[r, off["bselall"] + a:off["bselall"] + b] = one
    return _pack_words(blob)


def _in_maps(inputs):
    S = inputs["x"].shape[0]
    x = np.asarray(inputs["x"], np.float32)
    groups = _groups()
    blob = _blob_host(inputs)
    in_maps = []
    for d in range(NCORES):
        xl = x[:, groups[d], :]                       # (S, 8, 512)
        # xTp layout: (128 part, 4 k, S*BC) -> words
        xT = _bf16_u16(xl.transpose(2, 0, 1).reshape(4, 128, S * BC))
        xT = np.ascontiguousarray(xT.transpose(1, 0, 2))   # (128, 4, S*BC)
        in_maps.append({"blob": blob, "xTp": _pack_words(xT)})
    return in_maps


def _assemble(outs, S):
    groups = _groups()
    out = np.zeros((S, BATCH, 2 * HID), np.float32)
    for d in range(NCORES):
        raw = np.asarray(outs[d]["out"], np.float32)  # (S, 16, 512)
        G = groups[d]
        for b in range(BC):
            out[:, G[b], 0:HID] = raw[:, b, :]
            out[:, G[b], HID:] = raw[:, 8 + 7 - b, :]
    return out


class _Runner:
    """Caches the traced+compiled SPMD executable so repeat calls skip the
    (expensive) jax retrace and BIR re-serialization."""

    def __init__(self, S):
        import jax
        from jax.sharding import Mesh, PartitionSpec
        from jax.experimental.shard_map import shard_map
        from concourse import bass2jax
        from concourse.bass2jax import _bass_exec_p, partition_id_tensor

        bass2jax.install_neuronx_cc_hook()
        self.S = S
        nc = build_core_program(S)
        self.nc = nc
        partition_name = nc.partition_id_tensor.name if nc.partition_id_tensor else None
        in_names, out_names, out_avals, zero_outs = [], [], [], []
        for alloc in nc.m.functions[0].allocations:
            if not isinstance(alloc, mybir.MemoryLocationSet):
                continue
            name = alloc.memorylocations[0].name
            if alloc.kind == "ExternalInput":
                if name != partition_name:
                    in_names.append(name)
            elif alloc.kind == "ExternalOutput":
                shape = tuple(alloc.tensor_shape)
                dtype = mybir.dt.np(alloc.dtype)
                out_names.append(name)
                out_avals.append(jax.core.ShapedArray(shape, dtype))
                zero_outs.append(np.zeros(shape, dtype))
        n_params = len(in_names)
        self.in_names = list(in_names)
        self.out_names = out_names
        self.out_shapes = [tuple(a.shape) for a in out_avals]
        self.zero_outs = zero_outs
        all_in = in_names + out_names + ([partition_name] if partition_name else [])

        def _body(*args):
            operands = list(args)
            if partition_name is not None:
                operands.append(partition_id_tensor())
            return tuple(_bass_exec_p.bind(
                *operands,
                out_avals=tuple(out_avals),
                in_names=tuple(all_in),
                out_names=tuple(out_names),
                lowering_input_output_aliases=(),
                sim_require_finite=True,
                sim_require_nnan=True,
                nc=nc,
            ))

        devices = jax.devices()[:NCORES]
        mesh = Mesh(np.asarray(devices), ("core",))
        in_specs = (PartitionSpec("core"),) * (n_params + len(out_names))
        out_specs = (PartitionSpec("core"),) * len(out_names)
        self.fn = jax.jit(
            shard_map(_body, mesh=mesh, in_specs=in_specs,
                      out_specs=out_specs, check_rep=False),
            keep_unused=True)
        self.jax = jax

    def _zero_dev(self):
        # output buffers are fully overwritten by the kernel, so one
        # device-resident zero set can be reused across calls
        if not hasattr(self, "_zero_dev_bufs"):
            self._zero_dev_bufs = [
                self.jax.device_put(
                    np.zeros((NCORES * z.shape[0], *z.shape[1:]), z.dtype))
                for z in self.zero_outs]
        return self._zero_dev_bufs

    def run(self, in_maps):
        concat_in = [
            np.concatenate([np.asarray(m[nm]) for m in in_maps], axis=0)
            for nm in self.in_names]
        outs = self.fn(*concat_in, *self._zero_dev())
        return [
            {nm: np.asarray(outs[i]).reshape(NCORES, *self.out_shapes[i])[c]
             for i, nm in enumerate(self.out_names)}
            for c in range(NCORES)]

    def run_timed(self, in_maps, iters=5):
        """Stage inputs (and the pre-zeroed output buffers — every output
        element is written, so reuse is safe) on device; time executions."""
        import time
        concat_in = [
            self.jax.device_put(np.concatenate(
                [np.asarray(m[nm]) for m in in_maps], axis=0))
            for nm in self.in_names]
        concat_zero = self._zero_dev()
        o = self.fn(*concat_in, *concat_zero)
        self.jax.block_until_ready(o)
        best = float("inf")
        for _ in range(iters):
            t0 = time.perf_counter()
            o = self.fn(*concat_in, *concat_zero)
            self.jax.block_until_ready(o)
            best = min(best, time.perf_counter() - t0)
        return best


def kernel(**inputs):
    S = inputs["x"].shape[0]
    if S not in _CACHE:
        _CACHE[S] = _Runner(S)
    runner = _CACHE[S]
    outs = runner.run(_in_maps(inputs))
    return _assemble(outs, S)


if __name__ == "__main__":
    rng = np.random.default_rng(0)
    S = 32
    inputs = {"x": rng.standard_normal((S, 64, 512), dtype=np.float32)}
    s = 1.0 / np.sqrt(HID)
    u = lambda *shp: rng.uniform(-s, s, shp).astype(np.float32)
    for c, idim in (("f0", 512), ("b0", 512), ("f1", 1024), ("b1", 1024)):
        inputs[f"wih_{c}"] = u(G3, idim)
        inputs[f"whh_{c}"] = u(G3, HID)
        inputs[f"bih_{c}"] = u(G3)
        inputs[f"bhh_{c}"] = u(G3)
    out = kernel(**inputs)
    print("kernel ran, out", out.shape, float(np.abs(out).mean()))
